# revision 85
# baseline (speedup 1.0000x reference)
"""Trainium2 Bass kernel for nn_MultiHeadMLPAttentionModel.

Model: per (b, n) point: pairwise = [radar_b(4), pt(2)] (radar constant over n).
  h1 = relu(pairwise @ enc_w1 + enc_b1)            [B,N,64]
  pf = h1 @ enc_w2 + enc_b2                        [B,N,64]
  sh = relu(einsum('bnf,hfd', pairwise, sc_w1) + sc_b1)
  logits = einsum('bnhd,hd', sh, sc_w2) + sc_b2    [B,N,4]
  w = softmax(logits, axis=n)
  ctx = einsum('bnh,bnd', w, pf)  -> out MLP -> [B]

Key algebraic restructurings:
  * pooling commutes with the (linear) second encoder layer since softmax
    weights sum to 1:  ctx = (sum_n w * h1) @ enc_w2 + enc_b2.
  * sc_b2 is constant over n, so it drops out of the softmax.
  * the radar part of pairwise folds into per-b layer-1 bias vectors
    (computed on host).
  * softmax normalization is deferred: pooling accumulates unnormalized
    sum_n exp(l)*h1 and sum_n exp(l); division happens once per (b,h) after
    the (linear) enc2 matmul in phase D.

Slot-packing (the TRN2-specific trick): the PE HAM clock-gate only counts
matmuls with a large contraction dim as "busy" — K=4 matmuls run at the cold
1.2 GHz clock forever.  So every point-data matmul here uses K=128: batch b's
per-point features live on partition rows 8b..8b+6 of a shared [128, N]
tensor, and each per-b stationary is zero outside its slot rows.  Streamed
columns are unchanged; the whole kernel stays at 2.4 GHz.

Sharding: pure data parallel over B: 8 cores x 16 rows each.
"""

import numpy as np

import concourse.bass as bass
import concourse.tile as tile
from concourse import bacc, mybir

B, N, HID, HEADS = 128, 8192, 64, 4
NCORES = 8
BPC = B // NCORES  # 16 batch rows per core
CHUNK = 512
NCH = N // CHUNK  # 16
NB = N // 128  # 64 point-blocks of 128

F32 = mybir.dt.float32
BF16 = mybir.dt.bfloat16
FP8 = mybir.dt.float8e4
AF = mybir.ActivationFunctionType
ALU = mybir.AluOpType


def build_nc(reps=1, phases="APD", debug=False):
    from contextlib import ExitStack

    nc = bacc.Bacc()
    f32 = F32
    dbg_d = {}
    if debug:
        dbg_d["ctxnT"] = nc.dram_tensor("dbg_ctxnT", [65, 64], f32, kind="ExternalOutput")
        dbg_d["fct"] = nc.dram_tensor("dbg_fct", [64, 64], f32, kind="ExternalOutput")
        dbg_d["enm"] = nc.dram_tensor("dbg_enm", [128, NB * 64], BF16, kind="ExternalOutput")
        dbg_d["h1s"] = nc.dram_tensor("dbg_h1s", [128, 1024], BF16, kind="ExternalOutput")

    # xq: slot-packed points, row 8b+r = [xh, yh, xh, yh, 1, 1, 0, 0][r] of
    # batch b; col = position n
    xq_d = nc.dram_tensor("xq", [128, N], BF16, kind="ExternalInput")
    wpx_d = nc.dram_tensor("wpx", [128, BPC * 256], BF16, kind="ExternalInput")
    w2a_d = nc.dram_tensor("w2a", [128, BPC * 32], BF16, kind="ExternalInput")
    w2b_d = nc.dram_tensor("w2b", [128, BPC * 32], BF16, kind="ExternalInput")
    wex_d = nc.dram_tensor("wex", [128, BPC * 64], BF16, kind="ExternalInput")
    ew2b_d = nc.dram_tensor("ew2b", [65, 64], f32, kind="ExternalInput")
    ow1_d = nc.dram_tensor("ow1", [64, 256], f32, kind="ExternalInput")
    ob1_d = nc.dram_tensor("ob1", [1, 64], f32, kind="ExternalInput")
    w2o_d = nc.dram_tensor("w2o", [65, 1], f32, kind="ExternalInput")
    id64f_d = nc.dram_tensor("id64f", [64, 64], f32, kind="ExternalInput")
    on16_d = nc.dram_tensor("on16", [1, BPC], f32, kind="ExternalInput")
    out_d = nc.dram_tensor("out", [BPC], f32, kind="ExternalOutput")

    with tile.TileContext(nc) as tc, ExitStack() as ctx:
        consts = ctx.enter_context(tc.tile_pool(name="consts", bufs=1))

        def cload(dram, shape, nm, dt=f32):
            t = consts.tile(shape, dt, name=nm, tag=nm)
            nc.sync.dma_start(t[:], dram[:])
            return t

        wpx_s = cload(wpx_d, [128, BPC * 256], "wpx_s", BF16)
        xq_s = consts.tile([128, N], BF16, name="xq_s", tag="xq_s")
        nc.sync.dma_start(xq_s[:, 0:CHUNK], xq_d[:, 0:CHUNK])
        w2a_s = cload(w2a_d, [128, BPC * 32], "w2a_s", BF16)
        w2b_s = cload(w2b_d, [128, BPC * 32], "w2b_s", BF16)
        nc.sync.dma_start(xq_s[:, CHUNK : 4 * CHUNK], xq_d[:, CHUNK : 4 * CHUNK])
        nc.sync.dma_start(xq_s[:, 4 * CHUNK :], xq_d[:, 4 * CHUNK :])
        ew2b_s = cload(ew2b_d, [65, 64], "ew2b_s")
        ow1_s = cload(ow1_d, [64, 256], "ow1_s")
        ob1_s = cload(ob1_d, [1, 64], "ob1_s")
        w2o_s = cload(w2o_d, [65, 1], "w2o_s")
        on16_s = cload(on16_d, [1, BPC], "on16_s")

        wex_s = cload(wex_d, [128, BPC * 64], "wex_s", BF16)
        ones_s = consts.tile([128, 1], BF16, name="ones_s", tag="ones_s")
        nc.vector.memset(ones_s[:], 1.0)

        # n-major exp(logits): block t cols [t*64, (t+1)*64), within a block
        # partition p = n offset, col = 4*b + h
        enm = consts.tile([128, NB * 64], BF16, name="enm", tag="enm")
        ctxnT = consts.tile([65, 64], f32, name="ctxnT", tag="ctxnT")
        obuf = consts.tile([65, BPC], f32, name="obuf", tag="obuf")
        fct = consts.tile([64, 64], f32, name="fct", tag="fct")
        res = consts.tile([1, BPC], f32, name="res", tag="res")
        ones1r = consts.tile([1, 64], f32, name="ones1r", tag="ones1r")
        se_sb = consts.tile([1, 64], f32, name="se_sb", tag="se_sb")
        c1n = consts.tile([64, 64], f32, name="c1n", tag="c1n")
        id64f_s = cload(id64f_d, [64, 64], "id64f_s")
        # per-chunk sum-of-exp accumulators (from the exp activation) and
        # their reduction
        seacc = consts.tile([64, NCH], f32, name="seacc", tag="seacc")
        sejunk = consts.tile([64, NCH], f32, name="sejunk", tag="sejunk")
        secol = consts.tile([64, 1], f32, name="secol", tag="secol")
        nc.vector.memset(obuf[64:65, :], 1.0)
        nc.vector.memset(ones1r[:], 1.0)

        if "A" not in phases:
            nc.vector.memset(enm[:, 0:8], 0.0)
        for _rep in range(reps):
            _build_body(
                nc, tc, out_d,
                xq_s, wpx_s, wex_s, ones_s, w2a_s, w2b_s,
                ew2b_s, ow1_s, ob1_s, w2o_s, id64f_s, on16_s,
                enm, ctxnT, obuf, fct, res, ones1r, se_sb, c1n,
                seacc, sejunk, secol, phases,
                dbg_d,
            )
        if dbg_d:
            nc.sync.dma_start(dbg_d["ctxnT"][:], ctxnT[:])
            nc.sync.dma_start(dbg_d["fct"][:], fct[:])
            nc.sync.dma_start(dbg_d["enm"][:], enm[:])

    if not nc.is_finalized():
        nc.finalize()
    return nc


def _build_body(
    nc, tc, out_d,
    xq_s, wpx_s, wex_s, ones_s, w2a_s, w2b_s,
    ew2b_s, ow1_s, ob1_s, w2o_s, id64f_s, on16_s,
    enm, ctxnT, obuf, fct, res, ones1r, se_sb, c1n,
    seacc, sejunk, secol, phases="APD",
    dbg_d=None,
):
    dbg_d = dbg_d or {}
    from contextlib import ExitStack

    f32 = F32
    if "A" in phases:
        # ---- single interleaved pass: scores/logits for chunk c, then
        # exp/transpose + encoder/pooling for chunk c-1 ----
        with ExitStack() as pctx:
            shpool = pctx.enter_context(tc.tile_pool(name="shp", bufs=8))
            epool = pctx.enter_context(tc.tile_pool(name="ep", bufs=2))
            h1pool = pctx.enter_context(tc.tile_pool(name="h1p", bufs=3))
            psA = pctx.enter_context(tc.tile_pool(name="psA", bufs=4, space="PSUM"))
            psL = pctx.enter_context(tc.tile_pool(name="psL", bufs=1, space="PSUM"))
            psH = pctx.enter_context(tc.tile_pool(name="psH", bufs=2, space="PSUM"))
            psC = pctx.enter_context(tc.tile_pool(name="psC", bufs=1, space="PSUM"))

            DEPTH = 2  # software-pipeline depth: lg-MMs run DEPTH b's behind
            lg_done = {}
            # all-b context accumulator bank (hidden-major)
            cse_ps = psC.tile([64, 64], f32, name="cse_ps", tag="cse")
            hpend = []

            def expose(c):
                # exp of chunk c's logits, then XBAR DMA-transpose its 4
                # blocks n-major; accum_out collects the chunk's sum-of-exp
                lg = lg_done.pop(c)
                e_c = epool.tile([64, CHUNK], BF16, name="e_c", tag="e_c")
                nc.scalar.activation(
                    e_c[0:32, :], lg[0:32, :], AF.Exp,
                    accum_out=seacc[0:32, c : c + 1],
                )
                nc.scalar.activation(
                    e_c[32:64, :], lg[64:96, :], AF.Exp,
                    accum_out=seacc[32:64, c : c + 1],
                )
                for j in range(CHUNK // 128):
                    t = c * (CHUNK // 128) + j
                    nc.sync.dma_start_transpose(
                        enm[:, t * 64 : (t + 1) * 64],
                        e_c[:, j * 128 : (j + 1) * 128],
                    )

            def drain_pool():
                t, h1_sb = hpend.pop(0)
                # pooling: stationary = batch b's h1 block, moving = its 4
                # exp columns; accumulates into cse_ps[0:64, 4b:4b+4].
                # start=True resets has_written for the WHOLE bank, so only
                # the very first matmul carries it; later first-writes
                # overwrite via per-element has_written=0.
                for b in range(BPC):
                    nc.tensor.matmul(
                        cse_ps[:, 4 * b : 4 * b + 4],
                        h1_sb[:, b * 64 : (b + 1) * 64],
                        enm[:, t * 64 + 4 * b : t * 64 + 4 * b + 4],
                        start=(t == 0 and b == 0),
                        stop=(t == NB - 1),
                        skip_group_check=True,
                    )

            def ppartj(c, j):
                # encoder hidden + pooling for one 128-point block of chunk c
                t = c * (CHUNK // 128) + j
                xb = xq_s[:, t * 128 : (t + 1) * 128]
                h1_ps = [
                    psH.tile([128, 512], f32, name="h1_ps", tag="h1")
                    for _ in range(2)
                ]
                for g in range(2):
                    # K=64 slot-packed encoder matmul: 8 b's at once; the
                    # two halves run concurrently in the two PE row halves
                    lo = 64 * g
                    nc.tensor.matmul(
                        h1_ps[g][:],
                        xb[lo : lo + 64, :],
                        wex_s[lo : lo + 64, g * 512 : (g + 1) * 512],
                        start=True,
                        stop=True,
                        tile_position=(lo, 0),
                        skip_group_check=True,
                    )
                h1_sb = h1pool.tile([128, 1024], BF16, name="h1_sb", tag="h1s")
                for g in range(2):
                    dst = h1_sb[:, g * 512 : (g + 1) * 512]
                    if (2 * t + g) % 8 in (0, 3, 6):
                        nc.scalar.activation(dst, h1_ps[g][:], AF.Relu)
                    else:
                        nc.vector.tensor_scalar(
                            dst, h1_ps[g][:], 0.0, None, ALU.max
                        )
                hpend.append((t, h1_sb))
                if len(hpend) > 1:
                    drain_pool()

            def ppart(c):
                for j in range(CHUNK // 128):
                    ppartj(c, j)

            for c in range(NCH):
                xc = xq_s[:, c * CHUNK : (c + 1) * CHUNK]
                lg_ps = psL.tile([128, CHUNK], f32, name="lg_ps", tag="lg")
                pend = []
                dctr = [0]

                def drain_lg(lg_ps=lg_ps, dctr=dctr):
                    # logits for a (b, b+8) pair: the two b's write disjoint
                    # 32-row groups (rows 0:32 / 64:96), so their matmuls run
                    # concurrently in the two PE column halves
                    di = dctr[0]
                    dctr[0] += 1
                    pair = [pend.pop(0), pend.pop(0)]
                    for s in range(2):
                        w2 = w2a_s if s == 0 else w2b_s
                        for k, (b, sb) in enumerate(pair):
                            nc.tensor.matmul(
                                lg_ps[64 * k : 64 * k + 32, :],
                                w2[:, b * 32 : (b + 1) * 32],
                                sb[:, s * CHUNK : (s + 1) * CHUNK],
                                start=(di == 0 and s == 0),
                                stop=(di == BPC // 2 - 1 and s == 1),
                                tile_position=(0, 64 * k),
                                skip_group_check=True,
                            )

                for p in range(BPC // 2):
                    # K=64 slot-packed score matmuls: batches b<8 live in
                    # partition rows 0:64, b>=8 in rows 64:128, so the (p,
                    # p+8) pair runs concurrently in the two PE row halves.
                    # Bias rides in wpx slot rows 8b+4/5 (vs xq's ones rows),
                    # so both head-pair tiles of a b need just one bias-free
                    # relu over the 2-bank psum tile; fp8 output feeds the
                    # DoubleRow logits matmul.
                    bs = (p, p + 8)
                    sbt = [
                        shpool.tile([128, 2 * CHUNK], BF16, name="sh_sb", tag="shs")
                        for _ in range(2)
                    ]
                    pss = []
                    for u in range(2):
                        for k, b in enumerate(bs):
                            lo = 64 * k
                            ps = psA.tile([128, CHUNK], f32, name="sh_ps", tag="sh")
                            nc.tensor.matmul(
                                ps[:],
                                wpx_s[
                                    lo : lo + 64,
                                    b * 256 + u * 128 : b * 256 + (u + 1) * 128,
                                ],
                                xc[lo : lo + 64, :],
                                start=True,
                                stop=True,
                                tile_position=(lo, 0),
                                skip_group_check=True,
                            )
                            pss.append((u, k, ps))
                    for i, (u, k, ps) in enumerate(pss):
                        dst = sbt[k][:, u * CHUNK : (u + 1) * CHUNK]
                        # DVE is ~15% faster per relu than ACT; give it ~60%
                        if (4 * p + i) % 8 in (0, 3, 6):
                            nc.scalar.activation(dst, ps[:], AF.Relu)
                        else:
                            nc.vector.tensor_scalar(
                                dst, ps[:], 0.0, None, ALU.max
                            )
                    pend.append((bs[0], sbt[0]))
                    pend.append((bs[1], sbt[1]))
                    while len(pend) > 2 * DEPTH:
                        drain_lg()
                    # interleave chunk c-1's encoder/pool work between the
                    # score pairs to fill relu-latency bubbles
                    if c > 0 and p % 2 == 1:
                        if p == 1:
                            expose(c - 1)
                        ppartj(c - 1, p // 2)
                while pend:
                    drain_lg()
                lg_done[c] = lg_ps
            expose(NCH - 1)
            ppart(NCH - 1)
            while hpend:
                drain_pool()
            nc.vector.tensor_copy(out=ctxnT[0:64, :], in_=cse_ps[:])
            # reduce per-chunk sums-of-exp to a column
            nc.scalar.activation(
                sejunk[:], seacc[:], AF.Copy, accum_out=secol[:]
            )

    if "D" in phases:
        # ---- Phase D: pooled-context encoder layer 2 + output MLP --------
        with ExitStack() as pctx:
            psD = pctx.enter_context(tc.tile_pool(name="psD", bufs=1, space="PSUM"))
            # transpose the sum-of-exp column into ctxnT row 64
            serow_ps = psD.tile([1, 64], f32, name="serow_ps", tag="serow")
            nc.tensor.matmul(serow_ps[:], secol[:], id64f_s[:], start=True, stop=True)
            nc.vector.tensor_copy(out=ctxnT[64:65, :], in_=serow_ps[:])
            # fct_un[:, 4b+h] = sum_e * (enc_w2.T ctx_norm + enc_b2)
            fct_ps = psD.tile([64, 64], f32, name="fct_ps", tag="fctp")
            nc.tensor.matmul(fct_ps[:], ew2b_s[:], ctxnT[:], start=True, stop=True)
            # normalize columns by 1/sum_e via a rank-1 broadcast matmul
            nc.vector.reciprocal(se_sb[:], ctxnT[64:65, :])
            rbc_ps = psD.tile([64, 64], f32, name="rbc_ps", tag="rbcp")
            nc.tensor.matmul(rbc_ps[:], ones1r[:], se_sb[:], start=True, stop=True)
            nc.vector.tensor_copy(out=c1n[:], in_=rbc_ps[:])
            nc.vector.scalar_tensor_tensor(
                fct[:], fct_ps[:], 1.0, c1n[:], ALU.mult, ALU.mult
            )
            fct_bh = fct.rearrange("d (b h) -> d b h", h=HEADS)
            o1_ps = psD.tile([64, BPC], f32, name="o1_ps", tag="o1p")
            for h in range(HEADS):
                nc.tensor.matmul(
                    o1_ps[:],
                    ow1_s[:, h * 64 : (h + 1) * 64],
                    fct_bh[:, :, h],
                    start=(h == 0),
                    stop=False,
                    skip_group_check=True,
                )
            nc.tensor.matmul(
                o1_ps[:], ob1_s[:], on16_s[:], start=False, stop=True,
                skip_group_check=True,
            )
            nc.scalar.activation(obuf[0:64, :], o1_ps[:], AF.Relu)
            fin_ps = psD.tile([1, BPC], f32, name="fin_ps", tag="finp")
            nc.tensor.matmul(fin_ps[:], w2o_s[:], obuf[:], start=True, stop=True)
            nc.vector.tensor_copy(out=res[:], in_=fin_ps[:])
            nc.sync.dma_start(out_d.rearrange("(a n) -> a n", a=1), res[:])


def make_in_maps(inputs):
    """Host-side marshalling: slice B across cores and pack weights into the
    layouts the device program expects.

    bf16 note: the big streamed matmuls run in bf16.  To avoid systematic
    model-weight rounding, layer-1 weights are split hi/lo across extra
    contraction rows (w = hi + lo with both bf16); per-point input rounding
    is stochastic and averages out in the softmax pooling."""
    import ml_dtypes

    bf = ml_dtypes.bfloat16
    f = np.float32

    def split(a):
        hi = a.astype(bf)
        lo = (a - hi.astype(f)).astype(bf)
        return hi, lo
    radar = np.concatenate(
        [np.asarray(inputs["radar_xy"], f), np.asarray(inputs["radar_dir"], f)], axis=1
    )  # [B, 4]
    pts = np.asarray(inputs["pts"], f)
    enc_w1 = np.asarray(inputs["enc_w1"], f)
    enc_b1 = np.asarray(inputs["enc_b1"], f)
    enc_w2 = np.asarray(inputs["enc_w2"], f)
    enc_b2 = np.asarray(inputs["enc_b2"], f)
    sc_w1 = np.asarray(inputs["sc_w1"], f)
    sc_b1 = np.asarray(inputs["sc_b1"], f)
    sc_w2 = np.asarray(inputs["sc_w2"], f)
    out_w1 = np.asarray(inputs["out_w1"], f)
    out_b1 = np.asarray(inputs["out_b1"], f)
    out_w2 = np.asarray(inputs["out_w2"], f)
    out_b2 = np.asarray(inputs["out_b2"], f)

    # per-b layer-1 bias vectors (radar is constant over n)
    cb_sc = np.einsum("br,hrd->bhd", radar, sc_w1[:, :4, :]) + sc_b1  # [B, 4, 64]
    cb_enc = radar @ enc_w1[:4] + enc_b1  # [B, 64]

    # wp rows: [wxh, wyh, wxl, wyl] against xq rows [xh, yh, xh, yh]
    wp = np.empty((4, 256), bf)
    for h in range(HEADS):
        wxh, wxl = split(sc_w1[h, 4, :])
        wyh, wyl = split(sc_w1[h, 5, :])
        wp[0, h * 64 : (h + 1) * 64] = wxh
        wp[1, h * 64 : (h + 1) * 64] = wyh
        wp[2, h * 64 : (h + 1) * 64] = wxl
        wp[3, h * 64 : (h + 1) * 64] = wyl
    # heads 0,1 feed s1 (wp cols 0:128), heads 2,3 feed s2 (cols 128:256)

    # logits stationaries [128, 32] per b: local out-row 4*(b%8)+h; w2a
    # carries heads {0,1} (applied to the s1 half), w2b heads {2,3}
    w2a = np.zeros((128, BPC * 32), bf)
    w2b = np.zeros((128, BPC * 32), bf)
    for bl in range(BPC):
        loc = bl * 32 + 4 * (bl % 8)
        w2a[0:64, loc + 0] = sc_w2[0]
        w2a[64:128, loc + 1] = sc_w2[1]
        w2b[0:64, loc + 2] = sc_w2[2]
        w2b[64:128, loc + 3] = sc_w2[3]

    ew2b = np.concatenate([enc_w2, enc_b2[None, :]], axis=0)  # [65, 64]
    ow1 = np.empty((64, 256), f)
    for h in range(HEADS):
        ow1[:, h * 64 : (h + 1) * 64] = out_w1[h * 64 : (h + 1) * 64, :]
    ob1 = np.ascontiguousarray(out_b1[None, :])
    w2o = np.concatenate([out_w2, out_b2[None, :]], axis=0)  # [65, 1]
    id64f = np.eye(64, dtype=f)
    on16 = np.ones((1, BPC), f)

    exh, exl = split(enc_w1[4])
    eyh, eyl = split(enc_w1[5])

    in_maps = []
    for c in range(NCORES):
        sl = slice(c * BPC, (c + 1) * BPC)
        # slot-expanded score weights with the per-b radar-fold bias riding
        # in slot rows 8b+4/5 (hi/lo) against xq's ones rows
        wpx = np.zeros((128, BPC * 256), bf)
        for bl in range(BPC):
            wpx[8 * bl : 8 * bl + 4, bl * 256 : (bl + 1) * 256] = wp
            for u in range(2):
                vals = cb_sc[c * BPC + bl, 2 * u : 2 * u + 2].reshape(128)
                hi, lo = split(vals)
                cs = slice(bl * 256 + u * 128, bl * 256 + (u + 1) * 128)
                wpx[8 * bl + 4, cs] = hi
                wpx[8 * bl + 5, cs] = lo
        # wenm rows [wxh, wyh, wxl, wyl, bh, bl] vs xq rows [xh,yh,xh,yh,1,1]
        wenm = np.zeros((6, BPC * 64), bf)
        for bl in range(BPC):
            s = slice(bl * 64, (bl + 1) * 64)
            wenm[0, s] = exh
            wenm[1, s] = eyh
            wenm[2, s] = exl
            wenm[3, s] = eyl
            bh, blo = split(cb_enc[c * BPC + bl])
            wenm[4, s] = bh
            wenm[5, s] = blo
        # slot-expanded encoder weights: rows 8b..8b+6 at b's col range
        wex = np.zeros((128, BPC * 64), bf)
        for bl in range(BPC):
            wex[8 * bl : 8 * bl + 6, bl * 64 : (bl + 1) * 64] = wenm[
                :, bl * 64 : (bl + 1) * 64
            ]
        # slot-packed points [128, N]
        xq = np.zeros((128, N), bf)
        xh = pts[sl, :, 0].astype(bf)  # [BPC, N]
        yh = pts[sl, :, 1].astype(bf)
        for bl in range(BPC):
            xq[8 * bl + 0] = xh[bl]
            xq[8 * bl + 1] = yh[bl]
            xq[8 * bl + 2] = xh[bl]
            xq[8 * bl + 3] = yh[bl]
            xq[8 * bl + 4] = 1.0
            xq[8 * bl + 5] = 1.0
        in_maps.append(
            dict(
                xq=xq,
                wpx=wpx,
                w2a=w2a,
                w2b=w2b,
                wex=wex,
                ew2b=ew2b,
                ow1=ow1,
                ob1=ob1,
                w2o=w2o,
                id64f=id64f,
                on16=on16,
            )
        )
    return in_maps


_CACHE = {}


def _get_runner():
    """Build the Bass program once and a cached jitted PJRT executable over
    the 8 cores (shard_map along axis 0 of every input)."""
    if "runner" in _CACHE:
        return _CACHE["runner"]

    import jax
    from jax.sharding import Mesh, NamedSharding, PartitionSpec

    from concourse.bass2jax import (
        _bass_exec_p,
        install_neuronx_cc_hook,
        partition_id_tensor,
        shard_map,
    )

    nc = build_nc()
    _CACHE["nc"] = nc
    install_neuronx_cc_hook()
    partition_name = nc.partition_id_tensor.name if nc.partition_id_tensor else None
    in_names, out_names, out_avals = [], [], []
    for alloc in nc.m.functions[0].allocations:
        if not isinstance(alloc, mybir.MemoryLocationSet):
            continue
        name = alloc.memorylocations[0].name
        if alloc.kind == "ExternalInput":
            if name != partition_name:
                in_names.append(name)
        elif alloc.kind == "ExternalOutput":
            out_names.append(name)
            out_avals.append(
                jax.core.ShapedArray(tuple(alloc.tensor_shape), mybir.dt.np(alloc.dtype))
            )
    all_in_names = tuple(in_names + out_names)
    if partition_name is not None:
        all_in_names = all_in_names + (partition_name,)

    def _body(*args):
        operands = list(args)
        if partition_name is not None:
            operands.append(partition_id_tensor())
        return tuple(
            _bass_exec_p.bind(
                *operands,
                out_avals=tuple(out_avals),
                in_names=all_in_names,
                out_names=tuple(out_names),
                lowering_input_output_aliases=(),
                sim_require_finite=True,
                sim_require_nnan=True,
                nc=nc,
            )
        )

    devices = jax.devices()[:NCORES]
    mesh = Mesh(np.asarray(devices), ("core",))
    nin = len(in_names) + len(out_names)
    fn = jax.jit(
        shard_map(
            _body,
            mesh=mesh,
            in_specs=(PartitionSpec("core"),) * nin,
            out_specs=(PartitionSpec("core"),) * len(out_names),
            check_rep=False,
        ),
        keep_unused=True,
    )
    sharding = NamedSharding(mesh, PartitionSpec("core"))
    runner = (fn, sharding, in_names, out_avals)
    _CACHE["runner"] = runner
    return runner


def kernel(**inputs):
    import jax

    in_maps = make_in_maps(inputs)
    fn, sharding, in_names, out_avals = _get_runner()
    concat_in = [
        np.concatenate([np.asarray(in_maps[c][name]) for c in range(NCORES)], axis=0)
        for name in in_names
    ]
    concat_zeros = [
        np.zeros((NCORES * a.shape[0], *a.shape[1:]), a.dtype) for a in out_avals
    ]
    args = [jax.device_put(a, sharding) for a in (*concat_in, *concat_zeros)]
    (out,) = fn(*args)
    return np.asarray(out).reshape(B).astype(np.float32)


# revision 86
# speedup vs baseline: 1.3362x; 1.3362x over previous
"""Trainium2 Bass kernel for nn_MultiHeadMLPAttentionModel.

Model: per (b, n) point: pairwise = [radar_b(4), pt(2)] (radar constant over n).
  h1 = relu(pairwise @ enc_w1 + enc_b1)            [B,N,64]
  pf = h1 @ enc_w2 + enc_b2                        [B,N,64]
  sh = relu(einsum('bnf,hfd', pairwise, sc_w1) + sc_b1)
  logits = einsum('bnhd,hd', sh, sc_w2) + sc_b2    [B,N,4]
  w = softmax(logits, axis=n)
  ctx = einsum('bnh,bnd', w, pf)  -> out MLP -> [B]

Key algebraic restructurings:
  * pooling commutes with the (linear) second encoder layer since softmax
    weights sum to 1:  ctx = (sum_n w * h1) @ enc_w2 + enc_b2.
  * sc_b2 is constant over n, so it drops out of the softmax.
  * the radar part of pairwise folds into per-b layer-1 bias vectors
    (computed on host).
  * softmax normalization is deferred: pooling accumulates unnormalized
    sum_n exp(l)*h1 and sum_n exp(l); division happens once per (b,h) after
    the (linear) enc2 matmul in phase D.

Slot-packing (the TRN2-specific trick): the PE HAM clock-gate only counts
matmuls with a large contraction dim as "busy" — K=4 matmuls run at the cold
1.2 GHz clock forever.  So every point-data matmul here uses K=128: batch b's
per-point features live on partition rows 8b..8b+6 of a shared [128, N]
tensor, and each per-b stationary is zero outside its slot rows.  Streamed
columns are unchanged; the whole kernel stays at 2.4 GHz.

Sharding: pure data parallel over B: 8 cores x 16 rows each.
"""

import numpy as np

import concourse.bass as bass
import concourse.tile as tile
from concourse import bacc, mybir

B, N, HID, HEADS = 128, 8192, 64, 4
NCORES = 8
BPC = B // NCORES  # 16 batch rows per core
CHUNK = 512
NCH = N // CHUNK  # 16
NB = N // 128  # 64 point-blocks of 128

F32 = mybir.dt.float32
BF16 = mybir.dt.bfloat16
FP8 = mybir.dt.float8e4
AF = mybir.ActivationFunctionType
ALU = mybir.AluOpType


def build_nc(reps=1, phases="APD", debug=False):
    from contextlib import ExitStack

    nc = bacc.Bacc()
    f32 = F32
    dbg_d = {}
    if debug:
        dbg_d["ctxnT"] = nc.dram_tensor("dbg_ctxnT", [65, 64], f32, kind="ExternalOutput")
        dbg_d["fct"] = nc.dram_tensor("dbg_fct", [64, 64], f32, kind="ExternalOutput")
        dbg_d["enm"] = nc.dram_tensor("dbg_enm", [128, NB * 64], BF16, kind="ExternalOutput")
        dbg_d["h1s"] = nc.dram_tensor("dbg_h1s", [128, 1024], BF16, kind="ExternalOutput")

    # xq: slot-packed points, row 8b+r = [xh, yh, xh, yh, 1, 1, 0, 0][r] of
    # batch b; col = position n
    xq_d = nc.dram_tensor("xq", [128, N], BF16, kind="ExternalInput")
    wpx_d = nc.dram_tensor("wpx", [128, BPC * 256], BF16, kind="ExternalInput")
    w2a_d = nc.dram_tensor("w2a", [128, BPC * 32], BF16, kind="ExternalInput")
    w2b_d = nc.dram_tensor("w2b", [128, BPC * 32], BF16, kind="ExternalInput")
    wex_d = nc.dram_tensor("wex", [128, BPC * 64], BF16, kind="ExternalInput")
    ew2b_d = nc.dram_tensor("ew2b", [65, 64], f32, kind="ExternalInput")
    ow1_d = nc.dram_tensor("ow1", [64, 256], f32, kind="ExternalInput")
    ob1_d = nc.dram_tensor("ob1", [1, 64], f32, kind="ExternalInput")
    w2o_d = nc.dram_tensor("w2o", [65, 1], f32, kind="ExternalInput")
    id64f_d = nc.dram_tensor("id64f", [64, 64], f32, kind="ExternalInput")
    on16_d = nc.dram_tensor("on16", [1, BPC], f32, kind="ExternalInput")
    out_d = nc.dram_tensor("out", [BPC], f32, kind="ExternalOutput")

    with tile.TileContext(nc) as tc, ExitStack() as ctx:
        consts = ctx.enter_context(tc.tile_pool(name="consts", bufs=1))

        def cload(dram, shape, nm, dt=f32):
            t = consts.tile(shape, dt, name=nm, tag=nm)
            nc.sync.dma_start(t[:], dram[:])
            return t

        wpx_s = cload(wpx_d, [128, BPC * 256], "wpx_s", BF16)
        xq_s = consts.tile([128, N], BF16, name="xq_s", tag="xq_s")
        nc.sync.dma_start(xq_s[:, 0:CHUNK], xq_d[:, 0:CHUNK])
        w2a_s = cload(w2a_d, [128, BPC * 32], "w2a_s", BF16)
        w2b_s = cload(w2b_d, [128, BPC * 32], "w2b_s", BF16)
        nc.sync.dma_start(xq_s[:, CHUNK : 4 * CHUNK], xq_d[:, CHUNK : 4 * CHUNK])
        nc.sync.dma_start(xq_s[:, 4 * CHUNK :], xq_d[:, 4 * CHUNK :])
        ew2b_s = cload(ew2b_d, [65, 64], "ew2b_s")
        ow1_s = cload(ow1_d, [64, 256], "ow1_s")
        ob1_s = cload(ob1_d, [1, 64], "ob1_s")
        w2o_s = cload(w2o_d, [65, 1], "w2o_s")
        on16_s = cload(on16_d, [1, BPC], "on16_s")

        wex_s = cload(wex_d, [128, BPC * 64], "wex_s", BF16)
        ones_s = consts.tile([128, 1], BF16, name="ones_s", tag="ones_s")
        nc.vector.memset(ones_s[:], 1.0)

        # n-major exp(logits): block t cols [t*64, (t+1)*64), within a block
        # partition p = n offset, col = 4*b + h
        enm = consts.tile([128, NB * 64], BF16, name="enm", tag="enm")
        ctxnT = consts.tile([65, 64], f32, name="ctxnT", tag="ctxnT")
        obuf = consts.tile([65, BPC], f32, name="obuf", tag="obuf")
        fct = consts.tile([64, 64], f32, name="fct", tag="fct")
        res = consts.tile([1, BPC], f32, name="res", tag="res")
        ones1r = consts.tile([1, 64], f32, name="ones1r", tag="ones1r")
        se_sb = consts.tile([1, 64], f32, name="se_sb", tag="se_sb")
        c1n = consts.tile([64, 64], f32, name="c1n", tag="c1n")
        id64f_s = cload(id64f_d, [64, 64], "id64f_s")
        # per-chunk sum-of-exp accumulators (from the exp activation) and
        # their reduction
        seacc = consts.tile([64, NCH], f32, name="seacc", tag="seacc")
        sejunk = consts.tile([64, NCH], f32, name="sejunk", tag="sejunk")
        secol = consts.tile([64, 1], f32, name="secol", tag="secol")
        nc.vector.memset(obuf[64:65, :], 1.0)
        nc.vector.memset(ones1r[:], 1.0)

        if "A" not in phases:
            nc.vector.memset(enm[:, 0:8], 0.0)
        for _rep in range(reps):
            _build_body(
                nc, tc, out_d,
                xq_s, wpx_s, wex_s, ones_s, w2a_s, w2b_s,
                ew2b_s, ow1_s, ob1_s, w2o_s, id64f_s, on16_s,
                enm, ctxnT, obuf, fct, res, ones1r, se_sb, c1n,
                seacc, sejunk, secol, phases,
                dbg_d,
            )
        if dbg_d:
            nc.sync.dma_start(dbg_d["ctxnT"][:], ctxnT[:])
            nc.sync.dma_start(dbg_d["fct"][:], fct[:])
            nc.sync.dma_start(dbg_d["enm"][:], enm[:])

    if not nc.is_finalized():
        nc.finalize()
    return nc


def _build_body(
    nc, tc, out_d,
    xq_s, wpx_s, wex_s, ones_s, w2a_s, w2b_s,
    ew2b_s, ow1_s, ob1_s, w2o_s, id64f_s, on16_s,
    enm, ctxnT, obuf, fct, res, ones1r, se_sb, c1n,
    seacc, sejunk, secol, phases="APD",
    dbg_d=None,
):
    dbg_d = dbg_d or {}
    from contextlib import ExitStack

    f32 = F32
    if "A" in phases:
        # ---- single interleaved pass: scores/logits for chunk c, then
        # exp/transpose + encoder/pooling for chunk c-1 ----
        with ExitStack() as pctx:
            shpool = pctx.enter_context(tc.tile_pool(name="shp", bufs=8))
            epool = pctx.enter_context(tc.tile_pool(name="ep", bufs=2))
            h1pool = pctx.enter_context(tc.tile_pool(name="h1p", bufs=3))
            psA = pctx.enter_context(tc.tile_pool(name="psA", bufs=4, space="PSUM"))
            psL = pctx.enter_context(tc.tile_pool(name="psL", bufs=1, space="PSUM"))
            psH = pctx.enter_context(tc.tile_pool(name="psH", bufs=2, space="PSUM"))
            psC = pctx.enter_context(tc.tile_pool(name="psC", bufs=1, space="PSUM"))

            DEPTH = 2  # software-pipeline depth: lg-MMs run DEPTH b's behind
            lg_done = {}
            # all-b context accumulator bank (hidden-major)
            cse_ps = psC.tile([64, 64], f32, name="cse_ps", tag="cse")
            hpend = []

            def expose(c):
                # exp of chunk c's logits, then XBAR DMA-transpose its 4
                # blocks n-major; accum_out collects the chunk's sum-of-exp
                lg = lg_done.pop(c)
                e_c = epool.tile([64, CHUNK], BF16, name="e_c", tag="e_c")
                nc.scalar.activation(
                    e_c[0:32, :], lg[0:32, :], AF.Exp,
                    accum_out=seacc[0:32, c : c + 1],
                )
                nc.scalar.activation(
                    e_c[32:64, :], lg[64:96, :], AF.Exp,
                    accum_out=seacc[32:64, c : c + 1],
                )
                for j in range(CHUNK // 128):
                    t = c * (CHUNK // 128) + j
                    nc.sync.dma_start_transpose(
                        enm[:, t * 64 : (t + 1) * 64],
                        e_c[:, j * 128 : (j + 1) * 128],
                    )

            def drain_pool():
                t, h1_sb = hpend.pop(0)
                # pooling: stationary = batch b's h1 block, moving = its 4
                # exp columns; accumulates into cse_ps[0:64, 4b:4b+4].
                # start=True resets has_written for the WHOLE bank, so only
                # the very first matmul carries it; later first-writes
                # overwrite via per-element has_written=0.
                for b in range(BPC):
                    nc.tensor.matmul(
                        cse_ps[:, 4 * b : 4 * b + 4],
                        h1_sb[:, b * 64 : (b + 1) * 64],
                        enm[:, t * 64 + 4 * b : t * 64 + 4 * b + 4],
                        start=(t == 0 and b == 0),
                        stop=(t == NB - 1),
                        skip_group_check=True,
                    )

            def ppartj(c, j):
                # encoder hidden + pooling for one 128-point block of chunk c
                t = c * (CHUNK // 128) + j
                xb = xq_s[:, t * 128 : (t + 1) * 128]
                h1_ps = [
                    psH.tile([128, 512], f32, name="h1_ps", tag="h1")
                    for _ in range(2)
                ]
                for g in range(2):
                    # K=64 slot-packed encoder matmul: 8 b's at once; the
                    # two halves run concurrently in the two PE row halves
                    lo = 64 * g
                    nc.tensor.matmul(
                        h1_ps[g][:],
                        xb[lo : lo + 64, :],
                        wex_s[lo : lo + 64, g * 512 : (g + 1) * 512],
                        start=True,
                        stop=True,
                        tile_position=(lo, 0),
                        skip_group_check=True,
                    )
                h1_sb = h1pool.tile([128, 1024], BF16, name="h1_sb", tag="h1s")
                for g in range(2):
                    dst = h1_sb[:, g * 512 : (g + 1) * 512]
                    if g == t % 2:
                        nc.vector.tensor_scalar(
                            dst, h1_ps[g][:], 0.0, None, ALU.max
                        )
                    else:
                        nc.scalar.activation(dst, h1_ps[g][:], AF.Relu)
                hpend.append((t, h1_sb))
                if len(hpend) > 1:
                    drain_pool()

            def ppart(c):
                for j in range(CHUNK // 128):
                    ppartj(c, j)

            for c in range(NCH):
                xc = xq_s[:, c * CHUNK : (c + 1) * CHUNK]
                lg_ps = psL.tile([128, CHUNK], f32, name="lg_ps", tag="lg")
                pend = []
                dctr = [0]

                def drain_lg(lg_ps=lg_ps, dctr=dctr):
                    # logits for a (b, b+8) pair: the two b's write disjoint
                    # 32-row groups (rows 0:32 / 64:96), so their matmuls run
                    # concurrently in the two PE column halves
                    di = dctr[0]
                    dctr[0] += 1
                    pair = [pend.pop(0), pend.pop(0)]
                    for s in range(2):
                        w2 = w2a_s if s == 0 else w2b_s
                        for k, (b, sb) in enumerate(pair):
                            nc.tensor.matmul(
                                lg_ps[64 * k : 64 * k + 32, :],
                                w2[:, b * 32 : (b + 1) * 32],
                                sb[:, s * CHUNK : (s + 1) * CHUNK],
                                start=(di == 0 and s == 0),
                                stop=(di == BPC // 2 - 1 and s == 1),
                                tile_position=(0, 64 * k),
                                skip_group_check=True,
                            )

                for p in range(BPC // 2):
                    # K=64 slot-packed score matmuls: batches b<8 live in
                    # partition rows 0:64, b>=8 in rows 64:128, so the (p,
                    # p+8) pair runs concurrently in the two PE row halves.
                    # Bias rides in wpx slot rows 8b+4/5 (vs xq's ones rows),
                    # so both head-pair tiles of a b need just one bias-free
                    # relu over the 2-bank psum tile; fp8 output feeds the
                    # DoubleRow logits matmul.
                    bs = (p, p + 8)
                    sbt = [
                        shpool.tile([128, 2 * CHUNK], BF16, name="sh_sb", tag="shs")
                        for _ in range(2)
                    ]
                    pss = []
                    for u in range(2):
                        for k, b in enumerate(bs):
                            lo = 64 * k
                            ps = psA.tile([128, CHUNK], f32, name="sh_ps", tag="sh")
                            nc.tensor.matmul(
                                ps[:],
                                wpx_s[
                                    lo : lo + 64,
                                    b * 256 + u * 128 : b * 256 + (u + 1) * 128,
                                ],
                                xc[lo : lo + 64, :],
                                start=True,
                                stop=True,
                                tile_position=(lo, 0),
                                skip_group_check=True,
                            )
                            pss.append((u, k, ps))
                    for u, k, ps in pss:
                        dst = sbt[k][:, u * CHUNK : (u + 1) * CHUNK]
                        if u == k:
                            nc.scalar.activation(dst, ps[:], AF.Relu)
                        else:
                            nc.vector.tensor_scalar(
                                dst, ps[:], 0.0, None, ALU.max
                            )
                    pend.append((bs[0], sbt[0]))
                    pend.append((bs[1], sbt[1]))
                    while len(pend) > 2 * DEPTH:
                        drain_lg()
                    # interleave chunk c-1's encoder/pool work between the
                    # score pairs to fill relu-latency bubbles
                    if c > 0 and p % 2 == 1:
                        if p == 1:
                            expose(c - 1)
                        ppartj(c - 1, p // 2)
                while pend:
                    drain_lg()
                lg_done[c] = lg_ps
            expose(NCH - 1)
            ppart(NCH - 1)
            while hpend:
                drain_pool()
            nc.vector.tensor_copy(out=ctxnT[0:64, :], in_=cse_ps[:])
            # reduce per-chunk sums-of-exp to a column
            nc.scalar.activation(
                sejunk[:], seacc[:], AF.Copy, accum_out=secol[:]
            )

    if "D" in phases:
        # ---- Phase D: pooled-context encoder layer 2 + output MLP --------
        with ExitStack() as pctx:
            psD = pctx.enter_context(tc.tile_pool(name="psD", bufs=1, space="PSUM"))
            # transpose the sum-of-exp column into ctxnT row 64
            serow_ps = psD.tile([1, 64], f32, name="serow_ps", tag="serow")
            nc.tensor.matmul(serow_ps[:], secol[:], id64f_s[:], start=True, stop=True)
            nc.vector.tensor_copy(out=ctxnT[64:65, :], in_=serow_ps[:])
            # fct_un[:, 4b+h] = sum_e * (enc_w2.T ctx_norm + enc_b2)
            fct_ps = psD.tile([64, 64], f32, name="fct_ps", tag="fctp")
            nc.tensor.matmul(fct_ps[:], ew2b_s[:], ctxnT[:], start=True, stop=True)
            # normalize columns by 1/sum_e via a rank-1 broadcast matmul
            nc.vector.reciprocal(se_sb[:], ctxnT[64:65, :])
            rbc_ps = psD.tile([64, 64], f32, name="rbc_ps", tag="rbcp")
            nc.tensor.matmul(rbc_ps[:], ones1r[:], se_sb[:], start=True, stop=True)
            nc.vector.tensor_copy(out=c1n[:], in_=rbc_ps[:])
            nc.vector.scalar_tensor_tensor(
                fct[:], fct_ps[:], 1.0, c1n[:], ALU.mult, ALU.mult
            )
            fct_bh = fct.rearrange("d (b h) -> d b h", h=HEADS)
            o1_ps = psD.tile([64, BPC], f32, name="o1_ps", tag="o1p")
            for h in range(HEADS):
                nc.tensor.matmul(
                    o1_ps[:],
                    ow1_s[:, h * 64 : (h + 1) * 64],
                    fct_bh[:, :, h],
                    start=(h == 0),
                    stop=False,
                    skip_group_check=True,
                )
            nc.tensor.matmul(
                o1_ps[:], ob1_s[:], on16_s[:], start=False, stop=True,
                skip_group_check=True,
            )
            nc.scalar.activation(obuf[0:64, :], o1_ps[:], AF.Relu)
            fin_ps = psD.tile([1, BPC], f32, name="fin_ps", tag="finp")
            nc.tensor.matmul(fin_ps[:], w2o_s[:], obuf[:], start=True, stop=True)
            nc.vector.tensor_copy(out=res[:], in_=fin_ps[:])
            nc.sync.dma_start(out_d.rearrange("(a n) -> a n", a=1), res[:])


def make_in_maps(inputs):
    """Host-side marshalling: slice B across cores and pack weights into the
    layouts the device program expects.

    bf16 note: the big streamed matmuls run in bf16.  To avoid systematic
    model-weight rounding, layer-1 weights are split hi/lo across extra
    contraction rows (w = hi + lo with both bf16); per-point input rounding
    is stochastic and averages out in the softmax pooling."""
    import ml_dtypes

    bf = ml_dtypes.bfloat16
    f = np.float32

    def split(a):
        hi = a.astype(bf)
        lo = (a - hi.astype(f)).astype(bf)
        return hi, lo
    radar = np.concatenate(
        [np.asarray(inputs["radar_xy"], f), np.asarray(inputs["radar_dir"], f)], axis=1
    )  # [B, 4]
    pts = np.asarray(inputs["pts"], f)
    enc_w1 = np.asarray(inputs["enc_w1"], f)
    enc_b1 = np.asarray(inputs["enc_b1"], f)
    enc_w2 = np.asarray(inputs["enc_w2"], f)
    enc_b2 = np.asarray(inputs["enc_b2"], f)
    sc_w1 = np.asarray(inputs["sc_w1"], f)
    sc_b1 = np.asarray(inputs["sc_b1"], f)
    sc_w2 = np.asarray(inputs["sc_w2"], f)
    out_w1 = np.asarray(inputs["out_w1"], f)
    out_b1 = np.asarray(inputs["out_b1"], f)
    out_w2 = np.asarray(inputs["out_w2"], f)
    out_b2 = np.asarray(inputs["out_b2"], f)

    # per-b layer-1 bias vectors (radar is constant over n)
    cb_sc = np.einsum("br,hrd->bhd", radar, sc_w1[:, :4, :]) + sc_b1  # [B, 4, 64]
    cb_enc = radar @ enc_w1[:4] + enc_b1  # [B, 64]

    # wp rows: [wxh, wyh, wxl, wyl] against xq rows [xh, yh, xh, yh]
    wp = np.empty((4, 256), bf)
    for h in range(HEADS):
        wxh, wxl = split(sc_w1[h, 4, :])
        wyh, wyl = split(sc_w1[h, 5, :])
        wp[0, h * 64 : (h + 1) * 64] = wxh
        wp[1, h * 64 : (h + 1) * 64] = wyh
        wp[2, h * 64 : (h + 1) * 64] = wxl
        wp[3, h * 64 : (h + 1) * 64] = wyl
    # heads 0,1 feed s1 (wp cols 0:128), heads 2,3 feed s2 (cols 128:256)

    # logits stationaries [128, 32] per b: local out-row 4*(b%8)+h; w2a
    # carries heads {0,1} (applied to the s1 half), w2b heads {2,3}
    w2a = np.zeros((128, BPC * 32), bf)
    w2b = np.zeros((128, BPC * 32), bf)
    for bl in range(BPC):
        loc = bl * 32 + 4 * (bl % 8)
        w2a[0:64, loc + 0] = sc_w2[0]
        w2a[64:128, loc + 1] = sc_w2[1]
        w2b[0:64, loc + 2] = sc_w2[2]
        w2b[64:128, loc + 3] = sc_w2[3]

    ew2b = np.concatenate([enc_w2, enc_b2[None, :]], axis=0)  # [65, 64]
    ow1 = np.empty((64, 256), f)
    for h in range(HEADS):
        ow1[:, h * 64 : (h + 1) * 64] = out_w1[h * 64 : (h + 1) * 64, :]
    ob1 = np.ascontiguousarray(out_b1[None, :])
    w2o = np.concatenate([out_w2, out_b2[None, :]], axis=0)  # [65, 1]
    id64f = np.eye(64, dtype=f)
    on16 = np.ones((1, BPC), f)

    exh, exl = split(enc_w1[4])
    eyh, eyl = split(enc_w1[5])

    in_maps = []
    for c in range(NCORES):
        sl = slice(c * BPC, (c + 1) * BPC)
        # slot-expanded score weights with the per-b radar-fold bias riding
        # in slot rows 8b+4/5 (hi/lo) against xq's ones rows
        wpx = np.zeros((128, BPC * 256), bf)
        for bl in range(BPC):
            wpx[8 * bl : 8 * bl + 4, bl * 256 : (bl + 1) * 256] = wp
            for u in range(2):
                vals = cb_sc[c * BPC + bl, 2 * u : 2 * u + 2].reshape(128)
                hi, lo = split(vals)
                cs = slice(bl * 256 + u * 128, bl * 256 + (u + 1) * 128)
                wpx[8 * bl + 4, cs] = hi
                wpx[8 * bl + 5, cs] = lo
        # wenm rows [wxh, wyh, wxl, wyl, bh, bl] vs xq rows [xh,yh,xh,yh,1,1]
        wenm = np.zeros((6, BPC * 64), bf)
        for bl in range(BPC):
            s = slice(bl * 64, (bl + 1) * 64)
            wenm[0, s] = exh
            wenm[1, s] = eyh
            wenm[2, s] = exl
            wenm[3, s] = eyl
            bh, blo = split(cb_enc[c * BPC + bl])
            wenm[4, s] = bh
            wenm[5, s] = blo
        # slot-expanded encoder weights: rows 8b..8b+6 at b's col range
        wex = np.zeros((128, BPC * 64), bf)
        for bl in range(BPC):
            wex[8 * bl : 8 * bl + 6, bl * 64 : (bl + 1) * 64] = wenm[
                :, bl * 64 : (bl + 1) * 64
            ]
        # slot-packed points [128, N]
        xq = np.zeros((128, N), bf)
        xh = pts[sl, :, 0].astype(bf)  # [BPC, N]
        yh = pts[sl, :, 1].astype(bf)
        for bl in range(BPC):
            xq[8 * bl + 0] = xh[bl]
            xq[8 * bl + 1] = yh[bl]
            xq[8 * bl + 2] = xh[bl]
            xq[8 * bl + 3] = yh[bl]
            xq[8 * bl + 4] = 1.0
            xq[8 * bl + 5] = 1.0
        in_maps.append(
            dict(
                xq=xq,
                wpx=wpx,
                w2a=w2a,
                w2b=w2b,
                wex=wex,
                ew2b=ew2b,
                ow1=ow1,
                ob1=ob1,
                w2o=w2o,
                id64f=id64f,
                on16=on16,
            )
        )
    return in_maps


_CACHE = {}


def _get_runner():
    """Build the Bass program once and a cached jitted PJRT executable over
    the 8 cores (shard_map along axis 0 of every input)."""
    if "runner" in _CACHE:
        return _CACHE["runner"]

    import jax
    from jax.sharding import Mesh, NamedSharding, PartitionSpec

    from concourse.bass2jax import (
        _bass_exec_p,
        install_neuronx_cc_hook,
        partition_id_tensor,
        shard_map,
    )

    nc = build_nc()
    _CACHE["nc"] = nc
    install_neuronx_cc_hook()
    partition_name = nc.partition_id_tensor.name if nc.partition_id_tensor else None
    in_names, out_names, out_avals = [], [], []
    for alloc in nc.m.functions[0].allocations:
        if not isinstance(alloc, mybir.MemoryLocationSet):
            continue
        name = alloc.memorylocations[0].name
        if alloc.kind == "ExternalInput":
            if name != partition_name:
                in_names.append(name)
        elif alloc.kind == "ExternalOutput":
            out_names.append(name)
            out_avals.append(
                jax.core.ShapedArray(tuple(alloc.tensor_shape), mybir.dt.np(alloc.dtype))
            )
    all_in_names = tuple(in_names + out_names)
    if partition_name is not None:
        all_in_names = all_in_names + (partition_name,)

    def _body(*args):
        operands = list(args)
        if partition_name is not None:
            operands.append(partition_id_tensor())
        return tuple(
            _bass_exec_p.bind(
                *operands,
                out_avals=tuple(out_avals),
                in_names=all_in_names,
                out_names=tuple(out_names),
                lowering_input_output_aliases=(),
                sim_require_finite=True,
                sim_require_nnan=True,
                nc=nc,
            )
        )

    devices = jax.devices()[:NCORES]
    mesh = Mesh(np.asarray(devices), ("core",))
    nin = len(in_names) + len(out_names)
    fn = jax.jit(
        shard_map(
            _body,
            mesh=mesh,
            in_specs=(PartitionSpec("core"),) * nin,
            out_specs=(PartitionSpec("core"),) * len(out_names),
            check_rep=False,
        ),
        keep_unused=True,
    )
    sharding = NamedSharding(mesh, PartitionSpec("core"))
    runner = (fn, sharding, in_names, out_avals)
    _CACHE["runner"] = runner
    return runner


def kernel(**inputs):
    import jax

    in_maps = make_in_maps(inputs)
    fn, sharding, in_names, out_avals = _get_runner()
    concat_in = [
        np.concatenate([np.asarray(in_maps[c][name]) for c in range(NCORES)], axis=0)
        for name in in_names
    ]
    concat_zeros = [
        np.zeros((NCORES * a.shape[0], *a.shape[1:]), a.dtype) for a in out_avals
    ]
    args = [jax.device_put(a, sharding) for a in (*concat_in, *concat_zeros)]
    (out,) = fn(*args)
    return np.asarray(out).reshape(B).astype(np.float32)
